# revision 12
# baseline (speedup 1.0000x reference)
"""Trainium2 Bass kernel for the BiLSTM-CRF negative log-likelihood.

Strategy (8 NeuronCores, data-parallel over batch, 64 sequences/core):

The forward algorithm runs in *exp space*: the log-space recurrence
part_t = f_t + LSE_i(part_{t-1}[i] + trans[i,j]) becomes
p_t = (p_{t-1} @ exp(trans)) * exp(f_t - kappa) -- one small matmul plus one
elementwise multiply per step.  The constant shift kappa keeps magnitudes
centered; no per-step normalization is needed within bf16/f32 exponent range.

The chain over L=1024 is split into S=16 time segments of 64 steps.
exp(trans) is a strong contraction in the Hilbert projective metric, so each
segment recovers the true *direction* of the forward vector from an arbitrary
init after a short warmup (h=7); diagonal emission scaling does not affect
the contraction.  Per-segment telescoped log-gains and the per-sequence
stop-projection at t=len-1 are stitched into the exact log-partition.
Bookkeeping (stop-projection snapshot via a one-hot step mask, segment
boundary sum snapshots) rides through the same matmul as 6 extra state rows
per group.

State-row packing: emissions of both segment-parity groups sit contiguously
on partitions 0..103 (row 52g+j), bookkeeping on 104..115, zeros on 116..127.
The host supplies feats already *transposed* into this layout (pure integer
indexing / input relayout), so the per-chunk device prep is one DMA plus one
full-width Exp activation -- no on-device transpose at all.  Step-mask rows
stream from DRAM straight into partitions 104..127 of the emission tile.

The 1024 chain columns (16 segs x 64 batch) are split into two independent
column halves A/B pipelined across engines: while the PE multiplies half B,
the DVE applies half A's emission tile, so the per-step critical path is one
matmul + one multiply latency rather than their sum across the whole width.

Gold score: emission values feats[b,t,tags[b,t]] are host-GATHERED (integer
indexing / input layout; masked slots selected to zero), then summed on
device.  Transition score via host-side integer pair counts dotted against
transitions on DVE.  Per-core partial scalars are summed on the host.
"""

import contextlib

import numpy as np
import ml_dtypes

import concourse.bass as bass
import concourse.mybir as mybir
from concourse.bass_utils import run_bass_kernel_spmd
from concourse.tile import TileContext
from concourse.vector_clock import ScopedClock

BF16 = ml_dtypes.bfloat16

B, L, T = 512, 1024, 52
START, STOP = 50, 51
NCORES = 8
BS = B // NCORES          # 64 sequences per core
S = 16                    # time segments
ELL = L // S              # 64 owned steps per segment
H = 7                     # warmup steps
K = ELL + H + 1           # 72 recurrence steps
CH = 12                   # k-steps per prep chunk
NCH = K // CH             # 6 chunks
NP = 128                  # partitions
EMR = 104                 # emission rows: 52 states x 2 groups
BKR = 104                 # bookkeeping row base (6 per group)
GW = 512                  # chain columns per group (8 segs * 64 batch)
HW = GW // 2              # pipelined column half
CW = CH * GW              # free width of one prep chunk
KAPPA = float(np.log(T) + 0.5)
_SEG_OFF = [0] + [s * ELL - H for s in range(1, S)]
_K_START = [0] + [H] * (S - 1)


def _apply_tile_patch():
    """walrus here accepts only ONE sync-wait on CTRL-class (Drain/NoOp)
    instructions; Tile's end-of-kernel drain wants the whole global clock.
    Absorb the waits onto single-wait NOPs and mark them observed."""
    if getattr(TileContext, "_drain_patch_applied", False):
        return
    orig = TileContext._drain_and_barrier

    def patched(self, tick_clock, wait_clock):
        vclock = tick_clock.global_clock
        for i in range(len(vclock)):
            t = vclock[i]
            if t > 0:
                partial = ScopedClock()
                partial.require_at_least(None, i, t)
                nop_inst = self.nc.sync.nop()
                wait_clock.add_sem_waits(nop_inst.ins, partial)
        full = ScopedClock({None: vclock})
        for ec in wait_clock.engine_clocks:
            ec.update_past(full)
        orig(self, tick_clock, wait_clock)

    TileContext._drain_and_barrier = patched
    TileContext._drain_patch_applied = True


def _split_sync_waits(nc, maxw=1):
    """This walrus build rejects instructions carrying more than one sync
    wait.  Move excess waits onto same-engine NOPs inserted just before the
    instruction (semantically identical: the engine blocks either way)."""
    ctr = 0
    seen = set()
    for bb in nc.bb_map.values():
        inner = bb.bb if hasattr(bb, "bb") else bb
        if inner.name in seen:
            continue
        seen.add(inner.name)
        insts = list(inner.instructions)
        out = []
        for inst in insts:
            si = inst.sync_info
            if si is not None and si.on_wait and len(si.on_wait) > maxw:
                waits = list(si.on_wait)
                head, keep = waits[:-maxw], waits[-maxw:]
                for i in range(0, len(head), maxw):
                    nop = mybir.InstNoOp(name=f"I-wsplit-{ctr}", ins=[], outs=[])
                    ctr += 1
                    nop.engine = inst.engine
                    nop.sync_info = mybir.SyncInfo(
                        on_wait=head[i : i + maxw], on_update=[]
                    )
                    nc.register_instruction(nop)
                    out.append(nop)
                inst.sync_info = mybir.SyncInfo(
                    on_wait=keep, on_update=list(si.on_update or [])
                )
            out.append(inst)
        inner.instructions = out
    return nc


def _host_arrays(feats, transitions, mask, tags):
    lengths = mask.sum(axis=1).astype(np.int64)
    s_star = (lengths - 1) // ELL

    tags = tags.astype(np.int64)
    prev = np.concatenate(
        [np.full((B, 1), START, np.int64), tags[:, :-1]], axis=1
    )
    pair = prev * T + tags
    end_ids = np.take_along_axis(tags, (lengths - 1)[:, None], axis=1)[:, 0]

    # stationary matrix skeleton (emission blocks exp'd on device)
    etp = np.zeros((NP, NP), np.float32)
    for g in (0, 1):
        be, bb = 52 * g, BKR + 6 * g
        etp[bb + 0, bb + 1] = 1.0              # S -> A
        etp[bb + 1, bb + 1] = 1.0              # A -> A
        etp[be : be + 52, bb + 2] = 1.0        # sum p -> SE
        etp[bb + 2, bb + 3] = 1.0
        etp[bb + 3, bb + 3] = 1.0
        etp[be : be + 52, bb + 4] = 1.0        # sum p -> SS
        etp[bb + 4, bb + 5] = 1.0
        etp[bb + 5, bb + 5] = 1.0
    etp = etp.astype(BF16)

    # readout matrix: column 3g+r sums a bookkeeping row pair
    rmat = np.zeros((NP, 8), np.float32)
    for g in (0, 1):
        for r in range(3):
            lo = BKR + 6 * g + 2 * r
            rmat[lo, 3 * g + r] = 1.0
            rmat[lo + 1, 3 * g + r] = 1.0
    rmat = rmat.astype(BF16)

    # per-(k, seg) source time index, with the one-past-the-end slot of the
    # final segment clamped (consumed only by the final virtual step)
    t_idx = np.empty((S, K), np.int64)
    for s in range(S):
        t_idx[s] = _SEG_OFF[s] + np.arange(K)
    t_idx = np.clip(t_idx, 0, L - 1)

    # gathered gold emission values (host gather = input layout; masked
    # slots selected to zero; summation happens on device)
    gold_all = np.take_along_axis(feats, tags[..., None], axis=-1)[..., 0]
    gold_all = np.where(mask, gold_all, np.float32(0.0)).astype(np.float32)

    in_maps, host_ctx = [], []
    for c in range(NCORES):
        bsl = slice(c * BS, (c + 1) * BS)
        len_sh = lengths[bsl]
        sstar_sh = s_star[bsl]
        mask_sh = mask[bsl]

        # transposed feats: ftr[52g+j, (k, sp, h, b)] = feats[b, t(s,k), j]
        # with s = 8h + 2sp + g; chain col = sp*128 + 64h + b
        fc = feats[bsl][:, t_idx, :]           # [b, s, k, j]
        fc = fc.reshape(BS, 2, 4, 2, K, T)     # b, h, sp, g, k, j
        fc = fc.transpose(3, 5, 4, 2, 1, 0)    # g, j, k, sp, h, b
        ftr = np.ascontiguousarray(fc, np.float32).reshape(EMR, K * GW)

        # step-mask rows for partitions 104..127: per group
        # (d', 1, e_end, 1, e_start, 1); rows 12..23 stay zero
        dme = np.zeros((24, K, 4, 2, BS), np.float32)
        dme[1::6][:2], dme[3::6][:2], dme[5::6][:2] = 1.0, 1.0, 1.0
        for s in range(S):
            h, si = divmod(s, 8)
            g, sp = si % 2, si // 2
            ks = _K_START[s]
            dme[6 * g + 2, ks + ELL, sp, h, :] = 1.0       # e_end
            dme[6 * g + 4, ks, sp, h, :] = 1.0             # e_start
            fire = sstar_sh == s
            if fire.any():
                kf = len_sh[fire] - s * ELL + ks
                dme[6 * g + 0, kf, sp, h, np.where(fire)[0]] = 1.0
        dme = dme.reshape(24, K * GW).astype(BF16)

        # initial state [128, 512]
        pin = np.zeros((NP, 4, 2, BS), np.float32)
        for s in range(S):
            h, si = divmod(s, 8)
            g, sp = si % 2, si // 2
            if s == 0:
                pin[52 * g + START, sp, h, :] = 1.0
            else:
                pin[52 * g : 52 * g + 52, sp, h, :] = 1.0 / 52
        pin = pin.reshape(NP, GW).astype(BF16)

        hm = np.zeros((2, 3, 4, 2, BS), np.float32)
        for s in range(S):
            h, si = divmod(s, 8)
            g, sp = si % 2, si // 2
            end = (sstar_sh == s).astype(np.float32)
            sel = (sstar_sh > s).astype(np.float32)
            hm[g, 0, sp, h] = end
            hm[g, 1, sp, h] = sel
            hm[g, 2, sp, h] = -(sel + end)
        hm = hm.reshape(6, GW)

        cnt = np.bincount(pair[bsl][mask_sh].ravel(), minlength=T * T).astype(
            np.float32
        )
        cnt += np.bincount(end_ids[bsl] * T + STOP, minlength=T * T).astype(
            np.float32
        )

        # gold values laid out [p=(t%2)*64+b, c=t//2]
        gv = gold_all[bsl].reshape(BS, L // 2, 2).transpose(2, 0, 1)
        gv = np.ascontiguousarray(gv).reshape(NP, L // 2)

        in_maps.append(
            {
                "ftr": ftr,
                "dme": dme,
                "etp": etp,
                "pinit": pin,
                "hmask": hm,
                "counts": cnt.reshape(T, T),
                "trans": np.ascontiguousarray(transitions),
                "rmat": rmat,
                "gold": gv,
            }
        )
        host_ctx.append({"len_sum": int(len_sh.sum())})
    return in_maps, host_ctx


def _build_program(debug=False):
    nc = bass.Bass()
    dt = mybir.dt
    f32, bf = dt.float32, dt.bfloat16
    AF = mybir.ActivationFunctionType
    OP = mybir.AluOpType

    ftr_d = nc.declare_dram_parameter("ftr", [EMR, K * GW], f32, isOutput=False)
    dme_d = nc.declare_dram_parameter("dme", [24, K * GW], bf, isOutput=False)
    etp_d = nc.declare_dram_parameter("etp", [NP, NP], bf, isOutput=False)
    pin_d = nc.declare_dram_parameter("pinit", [NP, GW], bf, isOutput=False)
    hm_d = nc.declare_dram_parameter("hmask", [6, GW], f32, isOutput=False)
    cnt_d = nc.declare_dram_parameter("counts", [T, T], f32, isOutput=False)
    tr_d = nc.declare_dram_parameter("trans", [T, T], f32, isOutput=False)
    rm_d = nc.declare_dram_parameter("rmat", [NP, 8], bf, isOutput=False)
    gold_d = nc.declare_dram_parameter("gold", [NP, L // 2], f32, isOutput=False)
    out_d = nc.declare_dram_parameter("out", [128, 8], f32, isOutput=True)
    if debug:
        pd_d = nc.declare_dram_parameter("pdbg", [NP, GW], f32, isOutput=True)
        rd_d = nc.declare_dram_parameter("rdbg", [8, GW], f32, isOutput=True)

    # register float-const bias APs (only 0.0/1.0 exist by default)
    for val in (-KAPPA, 1e-20):
        t = nc.alloc_sbuf_tensor(f"const-f32-{val}", [128, 1], f32)
        nc.gpsimd.memset(t.ap(), val)
        nc.const_aps.aps[(f32, val)] = t.ap()
    nc.all_engine_barrier()

    with contextlib.ExitStack() as ctx, TileContext(nc) as tc:
        with (
            tc.tile_pool(name="const", bufs=1) as cpool,
            tc.tile_pool(name="ft", bufs=4) as fpool,
            tc.tile_pool(name="exsl", bufs=4) as xpool,
            tc.tile_pool(name="p", bufs=3) as ppool,
            tc.tile_pool(name="ps", bufs=2, space="PSUM") as pspool,
            tc.tile_pool(name="misc", bufs=1) as mpool,
        ):
            # ---- chunk-0 feats head first: the recurrence can't start
            # until exp(chunk0 head) lands, so its DMA leads the queue.
            # Small urgent transfers ride the sync HWDGE queue; all bulk
            # feats/mask traffic goes through the idle gpsimd SWDGE queue
            # so a slow bulk transfer never head-of-line blocks the ring.
            PARTS = 3
            PC = CW // PARTS  # columns per exp/DMA part (4 k-steps)
            ft0 = fpool.tile([EMR, CW], f32, tag="ft")
            nc.sync.dma_start(out=ft0[:, 0:PC], in_=ftr_d[:, 0:PC])

            # ---- constants
            trt = cpool.tile([T, T], f32, tag="tr")
            nc.sync.dma_start(out=trt[:], in_=tr_d[:])
            et = cpool.tile([NP, NP], bf, tag="et")
            nc.sync.dma_start(out=et[:], in_=etp_d[:])
            cntt = cpool.tile([T, T], f32, tag="cnt")
            nc.sync.dma_start(out=cntt[:], in_=cnt_d[:])
            hmt = cpool.tile([6, GW], f32, tag="hm")
            nc.sync.dma_start(out=hmt[:], in_=hm_d[:])
            rmt = cpool.tile([NP, 8], bf, tag="rm")
            nc.sync.dma_start(out=rmt[:], in_=rm_d[:])

            # exp(trans) in a base-0 scratch (compute ops need 32-aligned
            # partition bases), then DMA-copied into both diagonal blocks;
            # col 52 holds the stop-projection column exp(trans)[:, STOP]
            expb = mpool.tile([52, 53], bf, tag="expb")
            nc.scalar.activation(expb[:, 0:52], trt[:], AF.Exp)
            nc.vector.tensor_copy(
                expb[:, 52:53], expb[:, STOP : STOP + 1]
            )
            nc.sync.dma_start(out=et[0:52, 0:52], in_=expb[:, 0:52])
            nc.sync.dma_start(out=et[52:104, 52:104], in_=expb[:, 0:52])
            nc.sync.dma_start(
                out=et[0:52, BKR : BKR + 1], in_=expb[:, 52:53]
            )
            nc.sync.dma_start(
                out=et[52:104, BKR + 6 : BKR + 7], in_=expb[:, 52:53]
            )

            # ---- initial state, split into pipeline halves A/B
            p_cur = []
            for ci, cname in ((0, "A"), (HW, "B")):
                pt = ppool.tile([NP, HW], bf, tag=f"p{cname}")
                nc.sync.dma_start(out=pt[:], in_=pin_d[:, ci : ci + HW])
                p_cur.append(pt)

            # ---- emission prep, chunked over k; DMAs and Exps split into
            # 4-step parts so the recurrence unblocks as soon as each part
            # lands, and bulk transfers never gate unrelated small ones
            ex_slots = []
            for ck in range(NCH):
                if ck == 0:
                    ft = ft0
                else:
                    ft = fpool.tile([EMR, CW], f32, tag="ft")
                exsl = xpool.tile([NP, CW], bf, tag="exsl")
                ex_slots.append(exsl)
                nc.gpsimd.dma_start(
                    out=exsl[BKR:128, :],
                    in_=dme_d[:, ck * CW : (ck + 1) * CW],
                )
                for pp in range(PARTS):
                    lo, hi = pp * PC, (pp + 1) * PC
                    if ck > 0 or pp > 0:
                        nc.gpsimd.dma_start(
                            out=ft[:, lo:hi],
                            in_=ftr_d[:, ck * CW + lo : ck * CW + hi],
                        )
                    nc.scalar.activation(
                        exsl[0:EMR, lo:hi], ft[:, lo:hi], AF.Exp, bias=-KAPPA
                    )

            # ---- gold emission: host-gathered values, device sum (the DMA
            # launch is cheap; the reduce happens after the recurrence)
            gt = mpool.tile([NP, L // 2], f32, tag="gold")
            nc.gpsimd.dma_start(out=gt[:], in_=gold_d[:])

            # ---- recurrence, two pipelined column halves
            for k in range(K):
                ck, kk = divmod(k, CH)
                p_nxt = []
                for hi, (ci, cname) in enumerate(((0, "A"), (HW, "B"))):
                    ps = pspool.tile([NP, HW], f32, tag=f"ps{cname}")
                    nc.tensor.matmul(
                        ps[:], et[:], p_cur[hi][:], start=True, stop=True
                    )
                    pn = ppool.tile([NP, HW], bf, tag=f"p{cname}")
                    nc.vector.tensor_mul(
                        pn[:],
                        ps[:],
                        ex_slots[ck][:, kk * GW + ci : kk * GW + ci + HW],
                    )
                    p_nxt.append(pn)
                p_cur = p_nxt

            # ---- readout: one matmul per half sums bookkeeping row pairs
            rops = pspool.tile([8, GW], f32, tag="rops")
            for hi, ci in enumerate((0, HW)):
                nc.tensor.matmul(
                    rops[:, ci : ci + HW],
                    rmt[:],
                    p_cur[hi][:],
                    start=True,
                    stop=True,
                )
            lg = mpool.tile([6, GW], f32, tag="lg")
            nc.scalar.activation(lg[:], rops[0:6, :], AF.Ln, bias=1e-20)
            fprod = mpool.tile([6, GW], f32, tag="ftr")
            nc.vector.tensor_mul(fprod[:], lg[:], hmt[:])
            fwd_acc = mpool.tile([6, 1], f32, tag="fwa")
            nc.vector.tensor_reduce(
                fwd_acc[:], fprod[:], axis=mybir.AxisListType.X, op=OP.add
            )
            nc.sync.dma_start(out=out_d[0:6, 0:1], in_=fwd_acc[:])

            # ---- gold emission sum + trans-gold
            g_acc = mpool.tile([NP, 1], f32, tag="gacc")
            nc.vector.tensor_reduce(
                g_acc[:], gt[:], axis=mybir.AxisListType.X, op=OP.add
            )
            nc.sync.dma_start(out=out_d[:, 1:2], in_=g_acc[:])
            tg_prod = mpool.tile([T, T], f32, tag="tgt")
            nc.vector.tensor_mul(tg_prod[:], trt[:], cntt[:])
            tg_acc = mpool.tile([T, 1], f32, tag="tga")
            nc.vector.tensor_reduce(
                tg_acc[:], tg_prod[:], axis=mybir.AxisListType.X, op=OP.add
            )
            nc.sync.dma_start(out=out_d[0:T, 2:3], in_=tg_acc[:])

            if debug:
                pf = mpool.tile([NP, GW], f32, tag="pdbg")
                for hi, ci in enumerate((0, HW)):
                    nc.vector.tensor_copy(pf[:, ci : ci + HW], p_cur[hi][:])
                nc.sync.dma_start(out=pd_d[:], in_=pf[:])
                rf = mpool.tile([8, GW], f32, tag="rdbg")
                nc.scalar.activation(rf[:], rops[:], AF.Copy)
                nc.sync.dma_start(out=rd_d[:], in_=rf[:])
    _split_sync_waits(nc)
    return nc


_CACHE = {}


def kernel(feats, transitions, mask, tags):
    _apply_tile_patch()
    feats = np.asarray(feats, dtype=np.float32)
    transitions = np.asarray(transitions, dtype=np.float32)
    mask = np.asarray(mask).astype(bool)
    tags_in = np.asarray(tags).astype(np.int64)
    in_maps, host_ctx = _host_arrays(feats, transitions, mask, tags_in)

    if "nc" not in _CACHE:
        _CACHE["nc"] = _build_program()
    nc = _CACHE["nc"]

    res = run_bass_kernel_spmd(nc, in_maps, list(range(NCORES)))
    _CACHE["last_res"] = res

    total = 0.0
    for c in range(NCORES):
        out = np.asarray(res.results[c]["out"], dtype=np.float64)
        fwd = out[0:6, 0].sum() + KAPPA * host_ctx[c]["len_sum"]
        emit = out[:, 1].sum()
        tg = out[0:T, 2].sum()
        total += fwd - emit - tg
    return np.float32(total / B)


# revision 16
# speedup vs baseline: 1.0096x; 1.0096x over previous
"""Trainium2 Bass kernel for the BiLSTM-CRF negative log-likelihood.

Strategy (8 NeuronCores, data-parallel over batch, 64 sequences/core):

The forward algorithm runs in *exp space*: the log-space recurrence
part_t = f_t + LSE_i(part_{t-1}[i] + trans[i,j]) becomes
p_t = (p_{t-1} @ exp(trans)) * exp(f_t - kappa) -- one small matmul plus one
elementwise multiply per step.  The constant shift kappa keeps magnitudes
centered; no per-step normalization is needed within bf16/f32 exponent range.

The chain over L=1024 is split into S=16 time segments of 64 steps.
exp(trans) is a strong contraction in the Hilbert projective metric, so each
segment recovers the true *direction* of the forward vector from an arbitrary
init after a short warmup (h=7); diagonal emission scaling does not affect
the contraction.  Per-segment telescoped log-gains and the per-sequence
stop-projection at t=len-1 are stitched into the exact log-partition.
Bookkeeping (stop-projection snapshot via a one-hot step mask, segment
boundary sum snapshots) rides through the same matmul as 6 extra state rows
per group.

State-row packing: emissions of both segment-parity groups sit contiguously
on partitions 0..103 (row 52g+j), bookkeeping on 104..115, zeros on 116..127.
The host supplies feats already *transposed* into this layout (pure integer
indexing / input relayout), so the per-chunk device prep is one DMA plus one
full-width Exp activation -- no on-device transpose at all.  Step-mask rows
stream from DRAM straight into partitions 104..127 of the emission tile.

The 1024 chain columns (16 segs x 64 batch) are split into two independent
column halves A/B pipelined across engines: while the PE multiplies half B,
the DVE applies half A's emission tile, so the per-step critical path is one
matmul + one multiply latency rather than their sum across the whole width.

Gold score: emission values feats[b,t,tags[b,t]] are host-GATHERED (integer
indexing / input layout; masked slots selected to zero), then summed on
device.  Transition score via host-side integer pair counts dotted against
transitions on DVE.  Per-core partial scalars are summed on the host.
"""

import contextlib

import numpy as np
import ml_dtypes

import concourse.bass as bass
import concourse.mybir as mybir
from concourse.bass_utils import run_bass_kernel_spmd
from concourse.tile import TileContext
from concourse.vector_clock import ScopedClock

BF16 = ml_dtypes.bfloat16

B, L, T = 512, 1024, 52
START, STOP = 50, 51
NCORES = 8
BS = B // NCORES          # 64 sequences per core
S = 16                    # time segments
ELL = L // S              # 64 owned steps per segment
H = 7                     # warmup steps
K = ELL + H + 1           # 72 recurrence steps
CH = 12                   # k-steps per prep chunk
NCH = K // CH             # 6 chunks
NP = 128                  # partitions
EMR = 104                 # emission rows: 52 states x 2 groups
BKR = 104                 # bookkeeping row base (6 per group)
GW = 512                  # chain columns per group (8 segs * 64 batch)
HW = GW // 2              # pipelined column half
CW = CH * GW              # free width of one prep chunk
KAPPA = float(np.log(T) + 0.5)
_SEG_OFF = [0] + [s * ELL - H for s in range(1, S)]
_K_START = [0] + [H] * (S - 1)


def _apply_tile_patch():
    """walrus here accepts only ONE sync-wait on CTRL-class (Drain/NoOp)
    instructions; Tile's end-of-kernel drain wants the whole global clock.
    Absorb the waits onto single-wait NOPs and mark them observed."""
    if getattr(TileContext, "_drain_patch_applied", False):
        return
    orig = TileContext._drain_and_barrier

    def patched(self, tick_clock, wait_clock):
        vclock = tick_clock.global_clock
        for i in range(len(vclock)):
            t = vclock[i]
            if t > 0:
                partial = ScopedClock()
                partial.require_at_least(None, i, t)
                nop_inst = self.nc.sync.nop()
                wait_clock.add_sem_waits(nop_inst.ins, partial)
        full = ScopedClock({None: vclock})
        for ec in wait_clock.engine_clocks:
            ec.update_past(full)
        orig(self, tick_clock, wait_clock)

    TileContext._drain_and_barrier = patched
    TileContext._drain_patch_applied = True


def _split_sync_waits(nc, maxw=1):
    """This walrus build rejects instructions carrying more than one sync
    wait.  Move excess waits onto same-engine NOPs inserted just before the
    instruction (semantically identical: the engine blocks either way)."""
    ctr = 0
    seen = set()
    for bb in nc.bb_map.values():
        inner = bb.bb if hasattr(bb, "bb") else bb
        if inner.name in seen:
            continue
        seen.add(inner.name)
        insts = list(inner.instructions)
        out = []
        for inst in insts:
            si = inst.sync_info
            if si is not None and si.on_wait and len(si.on_wait) > maxw:
                waits = list(si.on_wait)
                head, keep = waits[:-maxw], waits[-maxw:]
                for i in range(0, len(head), maxw):
                    nop = mybir.InstNoOp(name=f"I-wsplit-{ctr}", ins=[], outs=[])
                    ctr += 1
                    nop.engine = inst.engine
                    nop.sync_info = mybir.SyncInfo(
                        on_wait=head[i : i + maxw], on_update=[]
                    )
                    nc.register_instruction(nop)
                    out.append(nop)
                inst.sync_info = mybir.SyncInfo(
                    on_wait=keep, on_update=list(si.on_update or [])
                )
            out.append(inst)
        inner.instructions = out
    return nc


def _host_arrays(feats, transitions, mask, tags):
    lengths = mask.sum(axis=1).astype(np.int64)
    s_star = (lengths - 1) // ELL

    tags = tags.astype(np.int64)
    prev = np.concatenate(
        [np.full((B, 1), START, np.int64), tags[:, :-1]], axis=1
    )
    pair = prev * T + tags
    end_ids = np.take_along_axis(tags, (lengths - 1)[:, None], axis=1)[:, 0]

    # stationary matrix skeleton (emission blocks exp'd on device)
    etp = np.zeros((NP, NP), np.float32)
    for g in (0, 1):
        be, bb = 52 * g, BKR + 6 * g
        etp[bb + 0, bb + 1] = 1.0              # S -> A
        etp[bb + 1, bb + 1] = 1.0              # A -> A
        etp[be : be + 52, bb + 2] = 1.0        # sum p -> SE
        etp[bb + 2, bb + 3] = 1.0
        etp[bb + 3, bb + 3] = 1.0
        etp[be : be + 52, bb + 4] = 1.0        # sum p -> SS
        etp[bb + 4, bb + 5] = 1.0
        etp[bb + 5, bb + 5] = 1.0
    etp = etp.astype(BF16)

    # readout matrix: column 3g+r sums a bookkeeping row pair
    rmat = np.zeros((NP, 8), np.float32)
    for g in (0, 1):
        for r in range(3):
            lo = BKR + 6 * g + 2 * r
            rmat[lo, 3 * g + r] = 1.0
            rmat[lo + 1, 3 * g + r] = 1.0
    rmat = rmat.astype(BF16)

    # per-(k, seg) source time index, with the one-past-the-end slot of the
    # final segment clamped (consumed only by the final virtual step)
    t_idx = np.empty((S, K), np.int64)
    for s in range(S):
        t_idx[s] = _SEG_OFF[s] + np.arange(K)
    t_idx = np.clip(t_idx, 0, L - 1)

    # gathered gold emission values (host gather = input layout; masked
    # slots selected to zero; summation happens on device)
    gold_all = np.take_along_axis(feats, tags[..., None], axis=-1)[..., 0]
    gold_all = np.where(mask, gold_all, np.float32(0.0)).astype(np.float32)

    in_maps, host_ctx = [], []
    for c in range(NCORES):
        bsl = slice(c * BS, (c + 1) * BS)
        len_sh = lengths[bsl]
        sstar_sh = s_star[bsl]
        mask_sh = mask[bsl]

        # transposed feats: ftr[52g+j, (k, sp, h, b)] = feats[b, t(s,k), j]
        # with s = 8h + 2sp + g; chain col = sp*128 + 64h + b
        fc = feats[bsl][:, t_idx, :]           # [b, s, k, j]
        fc = fc.reshape(BS, 2, 4, 2, K, T)     # b, h, sp, g, k, j
        fc = fc.transpose(3, 5, 4, 2, 1, 0)    # g, j, k, sp, h, b
        ftr = np.ascontiguousarray(fc, np.float32).reshape(EMR, K * GW)

        # step-mask rows for partitions 104..127: per group
        # (d', 1, e_end, 1, e_start, 1); rows 12..23 stay zero
        dme = np.zeros((24, K, 4, 2, BS), np.float32)
        dme[1::6][:2], dme[3::6][:2], dme[5::6][:2] = 1.0, 1.0, 1.0
        for s in range(S):
            h, si = divmod(s, 8)
            g, sp = si % 2, si // 2
            ks = _K_START[s]
            dme[6 * g + 2, ks + ELL, sp, h, :] = 1.0       # e_end
            dme[6 * g + 4, ks, sp, h, :] = 1.0             # e_start
            fire = sstar_sh == s
            if fire.any():
                kf = len_sh[fire] - s * ELL + ks
                dme[6 * g + 0, kf, sp, h, np.where(fire)[0]] = 1.0
        dme = dme.reshape(24, K * GW).astype(BF16)

        # initial state [128, 512]
        pin = np.zeros((NP, 4, 2, BS), np.float32)
        for s in range(S):
            h, si = divmod(s, 8)
            g, sp = si % 2, si // 2
            if s == 0:
                pin[52 * g + START, sp, h, :] = 1.0
            else:
                pin[52 * g : 52 * g + 52, sp, h, :] = 1.0 / 52
        pin = pin.reshape(NP, GW).astype(BF16)

        hm = np.zeros((2, 3, 4, 2, BS), np.float32)
        for s in range(S):
            h, si = divmod(s, 8)
            g, sp = si % 2, si // 2
            end = (sstar_sh == s).astype(np.float32)
            sel = (sstar_sh > s).astype(np.float32)
            hm[g, 0, sp, h] = end
            hm[g, 1, sp, h] = sel
            hm[g, 2, sp, h] = -(sel + end)
        hm = hm.reshape(6, GW)

        cnt = np.bincount(pair[bsl][mask_sh].ravel(), minlength=T * T).astype(
            np.float32
        )
        cnt += np.bincount(end_ids[bsl] * T + STOP, minlength=T * T).astype(
            np.float32
        )

        # gold values laid out [p=(t%2)*64+b, c=t//2]
        gv = gold_all[bsl].reshape(BS, L // 2, 2).transpose(2, 0, 1)
        gv = np.ascontiguousarray(gv).reshape(NP, L // 2)

        in_maps.append(
            {
                "ftr": ftr,
                "dme": dme,
                "etp": etp,
                "pinit": pin,
                "hmask": hm,
                "counts": cnt.reshape(T, T),
                "trans": np.ascontiguousarray(transitions),
                "rmat": rmat,
                "gold": gv,
            }
        )
        host_ctx.append({"len_sum": int(len_sh.sum())})
    return in_maps, host_ctx


def _build_program(debug=False):
    nc = bass.Bass()
    dt = mybir.dt
    f32, bf = dt.float32, dt.bfloat16
    AF = mybir.ActivationFunctionType
    OP = mybir.AluOpType

    ftr_d = nc.declare_dram_parameter("ftr", [EMR, K * GW], f32, isOutput=False)
    dme_d = nc.declare_dram_parameter("dme", [24, K * GW], bf, isOutput=False)
    etp_d = nc.declare_dram_parameter("etp", [NP, NP], bf, isOutput=False)
    pin_d = nc.declare_dram_parameter("pinit", [NP, GW], bf, isOutput=False)
    hm_d = nc.declare_dram_parameter("hmask", [6, GW], f32, isOutput=False)
    cnt_d = nc.declare_dram_parameter("counts", [T, T], f32, isOutput=False)
    tr_d = nc.declare_dram_parameter("trans", [T, T], f32, isOutput=False)
    rm_d = nc.declare_dram_parameter("rmat", [NP, 8], bf, isOutput=False)
    gold_d = nc.declare_dram_parameter("gold", [NP, L // 2], f32, isOutput=False)
    out_d = nc.declare_dram_parameter("out", [128, 8], f32, isOutput=True)
    if debug:
        pd_d = nc.declare_dram_parameter("pdbg", [NP, GW], f32, isOutput=True)
        rd_d = nc.declare_dram_parameter("rdbg", [8, GW], f32, isOutput=True)

    # register float-const bias APs (only 0.0/1.0 exist by default)
    for val in (-KAPPA, 1e-20):
        t = nc.alloc_sbuf_tensor(f"const-f32-{val}", [128, 1], f32)
        nc.gpsimd.memset(t.ap(), val)
        nc.const_aps.aps[(f32, val)] = t.ap()
    nc.all_engine_barrier()

    with contextlib.ExitStack() as ctx, TileContext(nc) as tc:
        with (
            tc.tile_pool(name="const", bufs=1) as cpool,
            tc.tile_pool(name="ft", bufs=4) as fpool,
            tc.tile_pool(name="exsl", bufs=4) as xpool,
            tc.tile_pool(name="p", bufs=3) as ppool,
            tc.tile_pool(name="ps", bufs=2, space="PSUM") as pspool,
            tc.tile_pool(name="misc", bufs=1) as mpool,
        ):
            # ---- chunk-0 feats head first: the recurrence can't start
            # until exp(chunk0 head) lands, so its DMA leads the queue.
            # Small urgent transfers ride the sync HWDGE queue; all bulk
            # feats/mask traffic goes through the idle gpsimd SWDGE queue
            # so a slow bulk transfer never head-of-line blocks the ring.
            PARTS = 3
            PC = CW // PARTS  # columns per exp/DMA part (4 k-steps)
            ft0 = fpool.tile([EMR, CW], f32, tag="ft")
            nc.sync.dma_start(out=ft0[:, 0:PC], in_=ftr_d[:, 0:PC])

            # ---- constants (pinit and the stationary matrix gate the
            # first matmul -- keep them at the head of the sync queue)
            trt = cpool.tile([T, T], f32, tag="tr")
            nc.sync.dma_start(out=trt[:], in_=tr_d[:])
            et = cpool.tile([NP, NP], bf, tag="et")
            nc.sync.dma_start(out=et[:], in_=etp_d[:])
            p_cur = []
            for ci, cname in ((0, "A"), (HW, "B")):
                pt = ppool.tile([NP, HW], bf, tag=f"p{cname}")
                nc.sync.dma_start(out=pt[:], in_=pin_d[:, ci : ci + HW])
                p_cur.append(pt)
            cntt = cpool.tile([T, T], f32, tag="cnt")
            nc.sync.dma_start(out=cntt[:], in_=cnt_d[:])
            hmt = cpool.tile([6, GW], f32, tag="hm")
            nc.sync.dma_start(out=hmt[:], in_=hm_d[:])
            rmt = cpool.tile([NP, 8], bf, tag="rm")
            nc.sync.dma_start(out=rmt[:], in_=rm_d[:])

            # exp(trans) in a base-0 scratch (compute ops need 32-aligned
            # partition bases), then DMA-copied into both diagonal blocks;
            # col 52 holds the stop-projection column exp(trans)[:, STOP]
            expb = mpool.tile([52, 53], bf, tag="expb")
            nc.scalar.activation(expb[:, 0:52], trt[:], AF.Exp)
            nc.scalar.activation(
                expb[:, 52:53], expb[:, STOP : STOP + 1], AF.Copy
            )
            nc.sync.dma_start(out=et[0:52, 0:52], in_=expb[:, 0:52])
            nc.sync.dma_start(out=et[52:104, 52:104], in_=expb[:, 0:52])
            nc.sync.dma_start(
                out=et[0:52, BKR : BKR + 1], in_=expb[:, 52:53]
            )
            nc.sync.dma_start(
                out=et[52:104, BKR + 6 : BKR + 7], in_=expb[:, 52:53]
            )

            # ---- emission prep, chunked over k; DMAs and Exps split into
            # 4-step parts so the recurrence unblocks as soon as each part
            # lands, and bulk transfers never gate unrelated small ones
            ex_slots = []
            for ck in range(NCH):
                if ck == 0:
                    ft = ft0
                else:
                    ft = fpool.tile([EMR, CW], f32, tag="ft")
                exsl = xpool.tile([NP, CW], bf, tag="exsl")
                ex_slots.append(exsl)
                nc.gpsimd.dma_start(
                    out=exsl[BKR:128, :],
                    in_=dme_d[:, ck * CW : (ck + 1) * CW],
                )
                for pp in range(PARTS):
                    lo, hi = pp * PC, (pp + 1) * PC
                    if ck > 0 or pp > 0:
                        # alternate bulk transfers between the sync HWDGE
                        # ring and the gpsimd SWDGE queue (~200 GB/s each)
                        eng = nc.sync if (ck * PARTS + pp) % 2 else nc.gpsimd
                        eng.dma_start(
                            out=ft[:, lo:hi],
                            in_=ftr_d[:, ck * CW + lo : ck * CW + hi],
                        )
                    nc.scalar.activation(
                        exsl[0:EMR, lo:hi], ft[:, lo:hi], AF.Exp, bias=-KAPPA
                    )

            # ---- gold emission: host-gathered values, device sum (the DMA
            # launch is cheap; the reduce happens after the recurrence)
            gt = mpool.tile([NP, L // 2], f32, tag="gold")
            nc.gpsimd.dma_start(out=gt[:], in_=gold_d[:])

            # ---- recurrence, two pipelined column halves
            for k in range(K):
                ck, kk = divmod(k, CH)
                p_nxt = []
                for hi, (ci, cname) in enumerate(((0, "A"), (HW, "B"))):
                    ps = pspool.tile([NP, HW], f32, tag=f"ps{cname}")
                    nc.tensor.matmul(
                        ps[:], et[:], p_cur[hi][:], start=True, stop=True
                    )
                    pn = ppool.tile([NP, HW], bf, tag=f"p{cname}")
                    nc.vector.tensor_mul(
                        pn[:],
                        ps[:],
                        ex_slots[ck][:, kk * GW + ci : kk * GW + ci + HW],
                    )
                    p_nxt.append(pn)
                p_cur = p_nxt

            # ---- readout: one matmul per half sums bookkeeping row pairs
            rops = pspool.tile([8, GW], f32, tag="rops")
            for hi, ci in enumerate((0, HW)):
                nc.tensor.matmul(
                    rops[:, ci : ci + HW],
                    rmt[:],
                    p_cur[hi][:],
                    start=True,
                    stop=True,
                )
            lg = mpool.tile([6, GW], f32, tag="lg")
            nc.scalar.activation(lg[:], rops[0:6, :], AF.Ln, bias=1e-20)
            fprod = mpool.tile([6, GW], f32, tag="ftr")
            nc.vector.tensor_mul(fprod[:], lg[:], hmt[:])
            fwd_acc = mpool.tile([6, 1], f32, tag="fwa")
            nc.vector.tensor_reduce(
                fwd_acc[:], fprod[:], axis=mybir.AxisListType.X, op=OP.add
            )
            nc.sync.dma_start(out=out_d[0:6, 0:1], in_=fwd_acc[:])

            # ---- gold emission sum + trans-gold
            g_acc = mpool.tile([NP, 1], f32, tag="gacc")
            nc.vector.tensor_reduce(
                g_acc[:], gt[:], axis=mybir.AxisListType.X, op=OP.add
            )
            nc.sync.dma_start(out=out_d[:, 1:2], in_=g_acc[:])
            tg_prod = mpool.tile([T, T], f32, tag="tgt")
            nc.vector.tensor_mul(tg_prod[:], trt[:], cntt[:])
            tg_acc = mpool.tile([T, 1], f32, tag="tga")
            nc.vector.tensor_reduce(
                tg_acc[:], tg_prod[:], axis=mybir.AxisListType.X, op=OP.add
            )
            nc.sync.dma_start(out=out_d[0:T, 2:3], in_=tg_acc[:])

            if debug:
                pf = mpool.tile([NP, GW], f32, tag="pdbg")
                for hi, ci in enumerate((0, HW)):
                    nc.vector.tensor_copy(pf[:, ci : ci + HW], p_cur[hi][:])
                nc.sync.dma_start(out=pd_d[:], in_=pf[:])
                rf = mpool.tile([8, GW], f32, tag="rdbg")
                nc.scalar.activation(rf[:], rops[:], AF.Copy)
                nc.sync.dma_start(out=rd_d[:], in_=rf[:])
    _split_sync_waits(nc)
    return nc


_CACHE = {}


def kernel(feats, transitions, mask, tags):
    _apply_tile_patch()
    feats = np.asarray(feats, dtype=np.float32)
    transitions = np.asarray(transitions, dtype=np.float32)
    mask = np.asarray(mask).astype(bool)
    tags_in = np.asarray(tags).astype(np.int64)
    in_maps, host_ctx = _host_arrays(feats, transitions, mask, tags_in)

    if "nc" not in _CACHE:
        _CACHE["nc"] = _build_program()
    nc = _CACHE["nc"]

    res = run_bass_kernel_spmd(nc, in_maps, list(range(NCORES)))
    _CACHE["last_res"] = res

    total = 0.0
    for c in range(NCORES):
        out = np.asarray(res.results[c]["out"], dtype=np.float64)
        fwd = out[0:6, 0].sum() + KAPPA * host_ctx[c]["len_sum"]
        emit = out[:, 1].sum()
        tg = out[0:T, 2].sum()
        total += fwd - emit - tg
    return np.float32(total / B)
